# revision 1
# baseline (speedup 1.0000x reference)
"""Trainium2 Bass kernel for CustomLSTM: B=64, T=1024, I=H=512.

Sharding: data-parallel over batch, 8 sequences per core on 8 cores.
Everything on-device lives in TRANSPOSED layout (hidden/gate dim on SBUF
partitions, batch on the free dim) so the per-step elementwise chain runs on
all 128 lanes and h^T feeds the next step's matmul directly, zero transposes.

Phase 1 (per core): xwT[g, t, b] = (x @ W + bias)^T via float32r matmuls
  (full-rate: moving free dim 512), staged to DRAM scratch.
Phase 2: 1024 sequential steps. gates^T = U-tiles (stationary, bf16)
  @ h^T (moving, N=8), PSUM-accumulated over 4 K-tiles; sigmoid/tanh on ACT,
  muls on DVE; h^T written back to SBUF state and staged out.
"""

import numpy as np
import ml_dtypes

B, T, I, H = 64, 1024, 512, 512
NC = 8            # cores
BL = B // NC      # 8 sequences per core
G4 = 4 * H        # 2048 gate dim
KT = I // 128     # 4 contraction tiles
MT = G4 // 128    # 16 gate m-tiles
C = T * BL        # 8192 columns, col = t*8 + b
MACRO = 64        # timesteps per For_i iteration
CHUNK = 64        # timesteps per precompute chunk (512 columns)


def build(nc, bass, tile, mybir):
    f32, bf16, f32r = mybir.dt.float32, mybir.dt.bfloat16, mybir.dt.float32r
    AF = mybir.ActivationFunctionType

    xT = nc.dram_tensor("xT", [128, KT, C], bf16, kind="ExternalInput")
    W = nc.dram_tensor("W", [128, KT, G4], bf16, kind="ExternalInput")
    U = nc.dram_tensor("U", [128, KT, G4], bf16, kind="ExternalInput")
    biasT = nc.dram_tensor("biasT", [128, MT], f32, kind="ExternalInput")
    hT_out = nc.dram_tensor("hT_out", [128, KT, C], f32, kind="ExternalOutput")

    with tile.TileContext(nc) as tc:
        with (
            tc.tile_pool(name="const", bufs=1) as const,
            tc.tile_pool(name="xtc", bufs=2) as xtc_pool,
            tc.tile_pool(name="xwc", bufs=3) as xwc_pool,
            tc.tile_pool(name="pre_ps", bufs=2, space="PSUM") as pre_ps,
            tc.tile_pool(name="state", bufs=1) as state,
            tc.tile_pool(name="xw", bufs=2) as xw_pool,
            tc.tile_pool(name="g_ps", bufs=2, space="PSUM") as g_ps,
            tc.tile_pool(name="work", bufs=2) as work,
            tc.tile_pool(name="stage", bufs=2) as stage_pool,
            tc.tile_pool(name="dram", bufs=1, space="DRAM") as dram,
        ):
            W_sb = const.tile([128, KT, G4], bf16)
            U_sb = const.tile([128, KT, G4], bf16)
            bias_sb = const.tile([128, MT], f32)
            nc.gpsimd.dma_start(W_sb[:], W[:])
            nc.gpsimd.dma_start(U_sb[:], U[:])
            nc.gpsimd.dma_start(bias_sb[:], biasT[:])

            xwT = dram.tile([128, MT, C], f32)

            # ---- Phase 1: xwT[:, m, t*8+b] = (x_t @ W + bias)^T ----
            for ch in range(T // CHUNK):
                cols = slice(ch * CHUNK * BL, (ch + 1) * CHUNK * BL)
                xtc = xtc_pool.tile([128, KT, CHUNK * BL], bf16)
                nc.gpsimd.dma_start(xtc[:], xT[:, :, cols])
                for m in range(MT):
                    ps = pre_ps.tile([128, CHUNK * BL], f32)
                    for k in range(KT):
                        nc.tensor.matmul(
                            ps[:],
                            W_sb[:, k, m * 128:(m + 1) * 128],
                            xtc[:, k, :],
                            start=(k == 0),
                            stop=(k == KT - 1),
                        )
                    xwc = xwc_pool.tile([128, CHUNK * BL], f32)
                    nc.scalar.activation(
                        xwc[:], ps[:], AF.Identity, bias=bias_sb[:, m:m + 1]
                    )
                    nc.gpsimd.dma_start(xwT[:, m, cols], xwc[:])

            # ---- Phase 2: recurrence ----
            hT = state.tile([128, KT * BL], bf16)   # col = k*8+b
            c_st = state.tile([128, KT * BL], f32)
            nc.vector.memset(hT[:], 0.0)
            nc.vector.memset(c_st[:], 0.0)

            def macro_body(c0, unroll):
                for u in range(unroll):
                    base = c0 + u * (MACRO * BL)
                    stage = stage_pool.tile([128, KT, MACRO * BL], f32)
                    xwm = xw_pool.tile([128, MT, MACRO * BL], f32)
                    nc.gpsimd.dma_start(
                        xwm[:], xwT[:, :, bass.ds(base, MACRO * BL)]
                    )
                    for s in range(MACRO):
                        xw = xwm[:, :, s * BL:(s + 1) * BL]
                        ps = g_ps.tile([128, MT * BL], f32)  # col = m*8+b
                        for m in range(MT):
                            for k in range(KT):
                                nc.tensor.matmul(
                                    ps[:, m * BL:(m + 1) * BL],
                                    U_sb[:, k, m * 128:(m + 1) * 128],
                                    hT[:, k * BL:(k + 1) * BL],
                                    start=(k == 0),
                                    stop=(k == KT - 1),
                                )
                        gs = work.tile([128, MT * BL], f32, tag="gs")
                        nc.vector.tensor_add(
                            gs[:].rearrange("p (m b) -> p m b", m=MT),
                            ps[:].rearrange("p (m b) -> p m b", m=MT),
                            xw,
                        )
                        act = work.tile([128, MT * BL], f32, tag="act")
                        # m 0-3=i, 4-7=f, 8-11=g, 12-15=o (cols of 32 each)
                        nc.scalar.activation(act[:, 0:64], gs[:, 0:64], AF.Sigmoid)
                        nc.scalar.activation(act[:, 64:96], gs[:, 64:96], AF.Tanh)
                        nc.scalar.activation(act[:, 96:128], gs[:, 96:128], AF.Sigmoid)
                        ig = work.tile([128, KT * BL], f32, tag="ig")
                        nc.vector.tensor_mul(ig[:], act[:, 0:32], act[:, 64:96])
                        nc.vector.tensor_mul(c_st[:], act[:, 32:64], c_st[:])
                        nc.vector.tensor_add(c_st[:], c_st[:], ig[:])
                        tc_t = work.tile([128, KT * BL], f32, tag="tc")
                        nc.scalar.activation(tc_t[:], c_st[:], AF.Tanh)
                        hslot = stage[:, :, s * BL:(s + 1) * BL]
                        nc.vector.tensor_mul(
                            hslot,
                            act[:, 96:128].rearrange("p (k b) -> p k b", k=KT),
                            tc_t[:].rearrange("p (k b) -> p k b", k=KT),
                        )
                        nc.vector.tensor_copy(
                            hT[:].rearrange("p (k b) -> p k b", k=KT), hslot
                        )
                    nc.gpsimd.dma_start(
                        hT_out[:, :, bass.ds(base, MACRO * BL)], stage[:]
                    )

            tc.For_i_unrolled_general(
                start=0, end=C, step=MACRO * BL,
                unrollable_body=macro_body, max_unroll=1,
                hint_engines=(mybir.EngineType.PE,),
            )
    nc.finalize()
    return nc


def kernel(x, W, U, bias):
    import concourse.bass as bass
    import concourse.bacc as bacc
    import concourse.tile as tile
    import concourse.mybir as mybir
    from concourse.bass_utils import run_bass_kernel_spmd

    x = np.asarray(x, np.float32)
    W = np.asarray(W, np.float32)
    U = np.asarray(U, np.float32)
    bias = np.asarray(bias, np.float32)

    nc = build(bacc.Bacc("TRN2", target_bir_lowering=False, num_devices=NC), bass, tile, mybir)

    Wt = np.ascontiguousarray(W.reshape(KT, 128, G4).transpose(1, 0, 2)).astype(ml_dtypes.bfloat16)
    Ut = np.ascontiguousarray(
        U.reshape(KT, 128, G4).transpose(1, 0, 2)
    ).astype(ml_dtypes.bfloat16)
    bt = np.ascontiguousarray(bias.reshape(MT, 128).T)

    in_maps = []
    for i in range(NC):
        xl = x[i * BL:(i + 1) * BL]                     # [8, 1024, 512]
        xTl = np.ascontiguousarray(
            xl.transpose(2, 1, 0).reshape(KT, 128, C)   # [512, T, 8]->[4,128,C]
        ).transpose(1, 0, 2)
        in_maps.append({
            "xT": np.ascontiguousarray(xTl).astype(ml_dtypes.bfloat16),
            "W": Wt, "U": Ut, "biasT": bt,
        })

    import os
    trace = bool(os.environ.get("LSTM_TRACE"))
    res = run_bass_kernel_spmd(
        nc, in_maps, core_ids=list(range(NC)), trace=trace
    )
    if trace and res.exec_time_ns is not None:
        print(f"HW exec time: {res.exec_time_ns} ns")
        print("trace:", (res.instructions_and_trace or (None, None))[1])
    out = np.empty((B, T, H), np.float32)
    for i in range(NC):
        ho = res.results[i]["hT_out"]                   # [128, 4, C]
        out[i * BL:(i + 1) * BL] = (
            ho.reshape(128, KT, T, BL).transpose(3, 2, 1, 0).reshape(BL, T, H)
        )
    return out



# revision 2
# speedup vs baseline: 1.2920x; 1.2920x over previous
"""Trainium2 Bass kernel for CustomLSTM: B=64, T=1024, I=H=512.

Sharding: data-parallel over batch, 8 sequences per core on 8 cores.
Everything on-device lives in TRANSPOSED layout (hidden/gate dim on SBUF
partitions, batch on the free dim) so the per-step elementwise chain runs on
all 128 lanes and h^T feeds the next step's matmul directly, zero transposes.

Phase 1 (per core): xwT[g, t, b] = (x @ W + bias)^T via bf16 matmuls
  (N=512 moving), bias folded into the PSUM->SBUF activation, staged to DRAM
  scratch in bf16.
Phase 2: 1024 sequential steps. Per step, 64 LDW+MM pairs (16 m-tiles x 4
  k-tiles, issue ~27ns/pair) into three gate-split PSUM tiles (i|f, g, o) so
  the elementwise chain for early gates overlaps the later gates' matmuls.
  h = sigma(o)*tanh(c) is written directly as bf16 into the staging tile,
  which doubles as the h state read by the next step's matmuls.
"""

import numpy as np
import ml_dtypes

B, T, I, H = 64, 1024, 512, 512
NC = 8            # cores
BL = B // NC      # 8 sequences per core
G4 = 4 * H        # 2048 gate dim
KT = I // 128     # 4 contraction tiles
MT = G4 // 128    # 16 gate m-tiles
C = T * BL        # 8192 columns, col = t*8 + b
MACRO = 64        # timesteps per For_i iteration
CHUNK = 64        # timesteps per precompute chunk (512 columns)


def build(nc, bass, tile, mybir):
    f32, bf16 = mybir.dt.float32, mybir.dt.bfloat16
    AF = mybir.ActivationFunctionType

    xT = nc.dram_tensor("xT", [128, KT, C], bf16, kind="ExternalInput")
    W = nc.dram_tensor("W", [128, KT, G4], bf16, kind="ExternalInput")
    U = nc.dram_tensor("U", [128, KT, G4], bf16, kind="ExternalInput")
    biasT = nc.dram_tensor("biasT", [128, MT], f32, kind="ExternalInput")
    hT_out = nc.dram_tensor("hT_out", [128, KT, C], bf16, kind="ExternalOutput")

    with tile.TileContext(nc) as tc:
        with (
            tc.tile_pool(name="const", bufs=1) as const,
            tc.tile_pool(name="xtc", bufs=2) as xtc_pool,
            tc.tile_pool(name="xwc", bufs=2) as xwc_pool,
            tc.tile_pool(name="pre_ps", bufs=2, space="PSUM") as pre_ps,
            tc.tile_pool(name="state", bufs=1) as state,
            tc.tile_pool(name="xw", bufs=2) as xw_pool,
            tc.tile_pool(name="ps_if", bufs=2, space="PSUM") as ps_if_pool,
            tc.tile_pool(name="ps_g", bufs=2, space="PSUM") as ps_g_pool,
            tc.tile_pool(name="ps_o", bufs=2, space="PSUM") as ps_o_pool,
            tc.tile_pool(name="work", bufs=2) as work,
            tc.tile_pool(name="stage", bufs=2) as stage_pool,
            tc.tile_pool(name="dram", bufs=1, space="DRAM") as dram,
        ):
            W_sb = const.tile([128, KT, G4], bf16)
            U_sb = const.tile([128, KT, G4], bf16)
            bias_sb = const.tile([128, MT], f32)
            nc.gpsimd.dma_start(W_sb[:], W[:])
            nc.gpsimd.dma_start(U_sb[:], U[:])
            nc.gpsimd.dma_start(bias_sb[:], biasT[:])

            xwT = dram.tile([128, MT, C], bf16)

            # ---- Phase 1: xwT[:, m, t*8+b] = (x_t @ W + bias)^T, bf16 ----
            for ch in range(T // CHUNK):
                cols = slice(ch * CHUNK * BL, (ch + 1) * CHUNK * BL)
                xtc = xtc_pool.tile([128, KT, CHUNK * BL], bf16)
                nc.gpsimd.dma_start(xtc[:], xT[:, :, cols])
                xwc = xwc_pool.tile([128, MT, CHUNK * BL], bf16)
                for m in range(MT):
                    ps = pre_ps.tile([128, CHUNK * BL], f32)
                    for k in range(KT):
                        nc.tensor.matmul(
                            ps[:],
                            W_sb[:, k, m * 128:(m + 1) * 128],
                            xtc[:, k, :],
                            start=(k == 0),
                            stop=(k == KT - 1),
                        )
                    nc.scalar.activation(
                        xwc[:, m, :], ps[:], AF.Identity, bias=bias_sb[:, m:m + 1]
                    )
                nc.gpsimd.dma_start(xwT[:, :, cols], xwc[:])

            # ---- Phase 2: recurrence ----
            # State: hT_st [128, k*8+b] bf16, c_st [128, k*8+b] f32
            hT_st = state.tile([128, KT * BL], bf16)
            c_st = state.tile([128, KT * BL], f32)
            nc.vector.memset(hT_st[:], 0.0)
            nc.vector.memset(c_st[:], 0.0)

            # m-tile gate map: i: 0-3, f: 4-7, g: 8-11, o: 12-15
            def macro_body(c0, unroll):
                for u in range(unroll):
                    base = c0 + u * (MACRO * BL)
                    stage = stage_pool.tile([128, KT, MACRO * BL], bf16)
                    xwm = xw_pool.tile([128, MT, MACRO * BL], bf16)
                    nc.gpsimd.dma_start(
                        xwm[:], xwT[:, :, bass.ds(base, MACRO * BL)]
                    )
                    for s in range(MACRO):
                        xw = xwm[:, :, s * BL:(s + 1) * BL]  # [128, 16, 8]
                        if s == 0:
                            h_prev = hT_st[:].rearrange("p (k b) -> p k b", k=KT)
                        else:
                            h_prev = stage[:, :, (s - 1) * BL:s * BL]
                        ps_if = ps_if_pool.tile([128, 8 * BL], f32)
                        ps_g = ps_g_pool.tile([128, 4 * BL], f32)
                        ps_o = ps_o_pool.tile([128, 4 * BL], f32)
                        # MM order: i, f, g, o. psum col = j*8+b (j = m % 4)
                        for m in range(MT):
                            if m < 8:
                                dst = ps_if[:, m * BL:(m + 1) * BL]
                            elif m < 12:
                                dst = ps_g[:, (m - 8) * BL:(m - 7) * BL]
                            else:
                                dst = ps_o[:, (m - 12) * BL:(m - 11) * BL]
                            for k in range(KT):
                                nc.tensor.matmul(
                                    dst,
                                    U_sb[:, k, m * 128:(m + 1) * 128],
                                    h_prev[:, k, :],
                                    start=(k == 0),
                                    stop=(k == KT - 1),
                                )
                        # ---- elementwise chain (DVE + ACT, overlaps PE) ----
                        gs_if = work.tile([128, 8 * BL], f32, tag="gs_if")
                        gs_g = work.tile([128, 4 * BL], f32, tag="gs_g")
                        gs_o = work.tile([128, 4 * BL], f32, tag="gs_o")
                        a_if = work.tile([128, 8 * BL], f32, tag="a_if")
                        tg = work.tile([128, 4 * BL], f32, tag="tg")
                        so = work.tile([128, 4 * BL], f32, tag="so")
                        thc = work.tile([128, 4 * BL], f32, tag="thc")
                        cf = work.tile([128, 4 * BL], f32, tag="cf")
                        ig = work.tile([128, 4 * BL], f32, tag="ig")

                        # DVE emit order matches data readiness
                        nc.vector.tensor_add(
                            gs_if[:].rearrange("p (m b) -> p m b", m=8),
                            ps_if[:].rearrange("p (m b) -> p m b", m=8),
                            xw[:, 0:8, :],
                        )
                        nc.vector.tensor_add(
                            gs_g[:].rearrange("p (m b) -> p m b", m=4),
                            ps_g[:].rearrange("p (m b) -> p m b", m=4),
                            xw[:, 8:12, :],
                        )
                        # ACT: sigmoid(i|f), tanh(g)
                        nc.scalar.activation(a_if[:], gs_if[:], AF.Sigmoid)
                        nc.scalar.activation(tg[:], gs_g[:], AF.Tanh)
                        # DVE: cf = sigmoid(f) * c
                        nc.vector.tensor_mul(cf[:], a_if[:, 32:64], c_st[:])
                        nc.vector.tensor_add(
                            gs_o[:].rearrange("p (m b) -> p m b", m=4),
                            ps_o[:].rearrange("p (m b) -> p m b", m=4),
                            xw[:, 12:16, :],
                        )
                        # DVE: ig = sigmoid(i) * tanh(g); c = cf + ig
                        nc.vector.tensor_mul(ig[:], a_if[:, 0:32], tg[:])
                        nc.vector.tensor_add(c_st[:], cf[:], ig[:])
                        # ACT: sigmoid(o), tanh(c)
                        nc.scalar.activation(so[:], gs_o[:], AF.Sigmoid)
                        nc.scalar.activation(thc[:], c_st[:], AF.Tanh)
                        # DVE: h = sigmoid(o) * tanh(c) -> bf16 stage slot
                        if s == MACRO - 1:
                            hdst = hT_st[:].rearrange("p (k b) -> p k b", k=KT)
                        else:
                            hdst = stage[:, :, s * BL:(s + 1) * BL]
                        nc.vector.tensor_mul(
                            hdst,
                            so[:].rearrange("p (k b) -> p k b", k=KT),
                            thc[:].rearrange("p (k b) -> p k b", k=KT),
                        )
                        if s == MACRO - 1:
                            nc.vector.tensor_copy(
                                stage[:, :, s * BL:(s + 1) * BL],
                                hT_st[:].rearrange("p (k b) -> p k b", k=KT),
                            )
                    nc.gpsimd.dma_start(
                        hT_out[:, :, bass.ds(base, MACRO * BL)], stage[:]
                    )

            tc.For_i_unrolled_general(
                start=0, end=C, step=MACRO * BL,
                unrollable_body=macro_body, max_unroll=1,
                hint_engines=(mybir.EngineType.PE,),
            )
    nc.finalize()
    return nc


def kernel(x, W, U, bias):
    import concourse.bass as bass
    import concourse.bacc as bacc
    import concourse.tile as tile
    import concourse.mybir as mybir
    from concourse.bass_utils import run_bass_kernel_spmd

    x = np.asarray(x, np.float32)
    W = np.asarray(W, np.float32)
    U = np.asarray(U, np.float32)
    bias = np.asarray(bias, np.float32)

    nc = build(bacc.Bacc("TRN2", target_bir_lowering=False, num_devices=NC), bass, tile, mybir)

    Wt = np.ascontiguousarray(W.reshape(KT, 128, G4).transpose(1, 0, 2)).astype(ml_dtypes.bfloat16)
    Ut = np.ascontiguousarray(
        U.reshape(KT, 128, G4).transpose(1, 0, 2)
    ).astype(ml_dtypes.bfloat16)
    bt = np.ascontiguousarray(bias.reshape(MT, 128).T)

    in_maps = []
    for i in range(NC):
        xl = x[i * BL:(i + 1) * BL]                     # [8, 1024, 512]
        xTl = np.ascontiguousarray(
            xl.transpose(2, 1, 0).reshape(KT, 128, C)   # [512, T, 8]->[4,128,C]
        ).transpose(1, 0, 2)
        in_maps.append({
            "xT": np.ascontiguousarray(xTl).astype(ml_dtypes.bfloat16),
            "W": Wt, "U": Ut, "biasT": bt,
        })

    import os
    trace = bool(os.environ.get("LSTM_TRACE"))
    res = run_bass_kernel_spmd(
        nc, in_maps, core_ids=list(range(NC)), trace=trace
    )
    if trace and res.exec_time_ns is not None:
        print(f"HW exec time: {res.exec_time_ns} ns")
        print("trace:", (res.instructions_and_trace or (None, None))[1])
    out = np.empty((B, T, H), np.float32)
    for i in range(NC):
        ho = np.asarray(res.results[i]["hT_out"]).astype(np.float32)  # [128, 4, C]
        out[i * BL:(i + 1) * BL] = (
            ho.reshape(128, KT, T, BL).transpose(3, 2, 1, 0).reshape(BL, T, H)
        )
    return out


# revision 7
# speedup vs baseline: 1.5386x; 1.1909x over previous
"""Trainium2 Bass kernel for CustomLSTM: B=64, T=1024, I=H=512.

Sharding: data-parallel over batch, 8 sequences per core on 8 cores.
Everything on-device lives in TRANSPOSED layout (hidden/gate dim on SBUF
partitions, batch on the free dim) so the per-step elementwise chain runs on
all 128 lanes and h^T feeds the next step's matmul directly, zero transposes.

Phase 1: xwT = (x @ W + bias)^T staged to DRAM scratch in bf16.
Phase 2: 1024 sequential steps, 128 per hardware-loop iteration.
  Per step: per-gate PSUM tiles are preloaded with xw by DVE; the 64 U
  matmuls accumulate on top (start=False, has_written set once by a warmup
  pass), so each sigmoid/tanh reads its PSUM tile directly after its gate's
  matmuls. Gate order g,f,i,o overlaps the elementwise chain with the PE
  phase; h is written as bf16 straight into the staging tile, which is also
  the h state consumed by the next step's matmuls.
"""

import numpy as np
import ml_dtypes

B, T, I, H = 64, 1024, 512, 512
NC = 8            # cores
BL = B // NC      # 8 sequences per core
G4 = 4 * H        # 2048 gate dim
KT = I // 128     # 4 contraction tiles
MT = G4 // 128    # 16 gate m-tiles
C = T * BL        # 8192 columns, col = t*8 + b
MACRO = 128       # timesteps per For_i iteration
HM = MACRO // 2   # half-macro (xwm double-buffer granularity)
CHUNK = 64        # timesteps per precompute chunk (512 columns)

# m-tile gate map in W/U column order: i: 0-3, f: 4-7, g: 8-11, o: 12-15
GATE_M = {"i": 0, "f": 4, "g": 8, "o": 12}


def build(nc, bass, tile, mybir):
    f32, bf16 = mybir.dt.float32, mybir.dt.bfloat16
    AF = mybir.ActivationFunctionType

    xT = nc.dram_tensor("xT", [128, KT, C], bf16, kind="ExternalInput")
    W = nc.dram_tensor("W", [128, KT, G4], bf16, kind="ExternalInput")
    U = nc.dram_tensor("U", [128, KT, G4], bf16, kind="ExternalInput")
    biasT = nc.dram_tensor("biasT", [128, MT], f32, kind="ExternalInput")
    ident = nc.dram_tensor("ident", [128, 128], bf16, kind="ExternalInput")
    hT_out = nc.dram_tensor("hT_out", [128, KT, C], bf16, kind="ExternalOutput")

    with tile.TileContext(nc) as tc:
        with (
            tc.tile_pool(name="const", bufs=1) as const,
            tc.tile_pool(name="xtc", bufs=2) as xtc_pool,
            tc.tile_pool(name="xwc", bufs=2) as xwc_pool,
            tc.tile_pool(name="state", bufs=1) as state,
            tc.tile_pool(name="work", bufs=2) as work,
            tc.tile_pool(name="dram", bufs=1, space="DRAM") as dram,
        ):
            W_sb = const.tile([128, KT, G4], bf16)
            U_sb = const.tile([128, KT, G4], bf16)
            bias_sb = const.tile([128, MT], f32)
            ident_sb = const.tile([128, 128], bf16)
            nc.gpsimd.dma_start(W_sb[:], W[:])
            nc.gpsimd.dma_start(U_sb[:], U[:])
            nc.gpsimd.dma_start(bias_sb[:], biasT[:])
            nc.gpsimd.dma_start(ident_sb[:], ident[:])

            # padded by HM*BL cols: last-iteration prefetch reads are in-range
            xwT = dram.tile([128, MT, C + HM * BL], bf16)

            # ---- Phase 1: xwT[:, m, t*8+b] = (x_t @ W + bias)^T, bf16 ----
            with tc.tile_pool(name="pre_ps", bufs=2, space="PSUM") as pre_ps:
                for ch in range(T // CHUNK):
                    cols = slice(ch * CHUNK * BL, (ch + 1) * CHUNK * BL)
                    xtc = xtc_pool.tile([128, KT, CHUNK * BL], bf16)
                    nc.gpsimd.dma_start(xtc[:], xT[:, :, cols])
                    xwc = xwc_pool.tile([128, MT, CHUNK * BL], bf16)
                    for m in range(MT):
                        ps = pre_ps.tile([128, CHUNK * BL], f32)
                        for k in range(KT):
                            nc.tensor.matmul(
                                ps[:],
                                W_sb[:, k, m * 128:(m + 1) * 128],
                                xtc[:, k, :],
                                start=(k == 0),
                                stop=(k == KT - 1),
                            )
                        nc.scalar.activation(
                            xwc[:, m, :], ps[:], AF.Identity,
                            bias=bias_sb[:, m:m + 1],
                        )
                    nc.gpsimd.dma_start(xwT[:, :, cols], xwc[:])

            # ---- Phase 2 ----
            with (
                tc.tile_pool(name="ps_g", bufs=1, space="PSUM") as ps_g_pool,
                tc.tile_pool(name="ps_f", bufs=1, space="PSUM") as ps_f_pool,
                tc.tile_pool(name="ps_i", bufs=1, space="PSUM") as ps_i_pool,
                tc.tile_pool(name="ps_o", bufs=1, space="PSUM") as ps_o_pool,
                tc.tile_pool(name="a_f", bufs=1, space="PSUM") as a_f_pool,
                tc.tile_pool(name="a_i", bufs=1, space="PSUM") as a_i_pool,
                tc.tile_pool(name="thc", bufs=1, space="PSUM") as thc_pool,
            ):
                GB = 4 * BL  # 32 cols per gate tile
                hT_st = state.tile([128, KT * BL], bf16)
                c_st = state.tile([128, KT * BL], f32)
                stage = state.tile([128, KT, MACRO * BL], bf16)
                xwmA = state.tile([128, MT, HM * BL], bf16)
                xwmB = state.tile([128, MT, HM * BL], bf16)
                nc.vector.memset(hT_st[:], 0.0)
                nc.vector.memset(c_st[:], 0.0)
                nc.gpsimd.dma_start(xwmA[:], xwT[:, :, 0:HM * BL])

                ps_g = ps_g_pool.tile([128, GB], f32)
                ps_f = ps_f_pool.tile([128, GB], f32)
                ps_i = ps_i_pool.tile([128, GB], f32)
                ps_o = ps_o_pool.tile([128, GB], f32)
                PS = {"g": ps_g, "f": ps_f, "i": ps_i, "o": ps_o}

                def mm_group(gate, h_prev, xwm, slot):
                    m0 = GATE_M[gate]
                    dst = PS[gate]
                    # inject xw via identity matmul (start=True), then
                    # accumulate the 16 U matmuls on top.
                    nc.tensor.matmul(
                        dst[:].rearrange("p (m b) -> p m b", m=4),
                        ident_sb[:],
                        xwm[:, m0:m0 + 4, slot * BL:(slot + 1) * BL],
                        start=True,
                        stop=False,
                        skip_group_check=True,
                    )
                    for j in range(4):
                        m = m0 + j
                        for k in range(KT):
                            nc.tensor.matmul(
                                dst[:, j * BL:(j + 1) * BL],
                                U_sb[:, k, m * 128:(m + 1) * 128],
                                h_prev[:, k, :],
                                start=False,
                                stop=(k == KT - 1),
                                skip_group_check=True,
                            )

                def macro_body(c0, unroll):
                    assert unroll == 1
                    # prefetch second half of this iteration's xw
                    nc.gpsimd.dma_start(
                        xwmB[:], xwT[:, :, bass.ds(c0 + HM * BL, HM * BL)]
                    )
                    for s in range(MACRO):
                        if s == 0:
                            h_prev = hT_st[:].rearrange("p (k b) -> p k b", k=KT)
                        else:
                            h_prev = stage[:, :, (s - 1) * BL:s * BL]
                        xwm, slot = (xwmA, s) if s < HM else (xwmB, s - HM)
                        for gate in ("g", "f", "i", "o"):
                            mm_group(gate, h_prev, xwm, slot)
                        tg = work.tile([128, GB], f32, tag="tg")
                        so = work.tile([128, GB], f32, tag="so")
                        cf = work.tile([128, GB], f32, tag="cf")
                        ig = work.tile([128, GB], f32, tag="ig")
                        a_f = a_f_pool.tile([128, GB], f32)
                        a_i = a_i_pool.tile([128, GB], f32)
                        thc = thc_pool.tile([128, GB], f32)

                        nc.scalar.activation(tg[:], ps_g[:], AF.Tanh)
                        nc.scalar.activation(a_f[:], ps_f[:], AF.Sigmoid)
                        nc.vector.tensor_mul(cf[:], a_f[:], c_st[:])
                        nc.scalar.activation(a_i[:], ps_i[:], AF.Sigmoid)
                        nc.vector.tensor_mul(ig[:], a_i[:], tg[:])
                        nc.vector.tensor_add(c_st[:], cf[:], ig[:])
                        nc.scalar.activation(so[:], ps_o[:], AF.Sigmoid)
                        nc.scalar.activation(thc[:], c_st[:], AF.Tanh)
                        if s == MACRO - 1:
                            hdst = hT_st[:].rearrange("p (k b) -> p k b", k=KT)
                        else:
                            hdst = stage[:, :, s * BL:(s + 1) * BL]
                        nc.vector.tensor_mul(
                            hdst,
                            so[:].rearrange("p (k b) -> p k b", k=KT),
                            thc[:].rearrange("p (k b) -> p k b", k=KT),
                        )
                        if s == HM - 1:
                            # first half done: stage out, prefetch next
                            # iteration's first-half xw
                            nc.gpsimd.dma_start(
                                hT_out[:, :, bass.ds(c0, HM * BL)],
                                stage[:, :, 0:HM * BL],
                            )
                            nc.gpsimd.dma_start(
                                xwmA[:],
                                xwT[:, :, bass.ds(c0 + MACRO * BL, HM * BL)],
                            )
                        elif s == MACRO - 1:
                            nc.vector.tensor_copy(
                                stage[:, :, s * BL:(s + 1) * BL],
                                hT_st[:].rearrange("p (k b) -> p k b", k=KT),
                            )
                            nc.gpsimd.dma_start(
                                hT_out[:, :, bass.ds(c0 + HM * BL, HM * BL)],
                                stage[:, :, HM * BL:MACRO * BL],
                            )

                tc.For_i_unrolled_general(
                    start=0, end=C, step=MACRO * BL,
                    unrollable_body=macro_body, max_unroll=1,
                    hint_engines=(mybir.EngineType.PE,),
                )
    nc.finalize()
    return nc


def kernel(x, W, U, bias):
    import concourse.bass as bass
    import concourse.bacc as bacc
    import concourse.tile as tile
    import concourse.mybir as mybir
    from concourse.bass_utils import run_bass_kernel_spmd

    x = np.asarray(x, np.float32)
    W = np.asarray(W, np.float32)
    U = np.asarray(U, np.float32)
    bias = np.asarray(bias, np.float32)

    nc = build(bacc.Bacc("TRN2", target_bir_lowering=False, num_devices=NC), bass, tile, mybir)

    Wt = np.ascontiguousarray(W.reshape(KT, 128, G4).transpose(1, 0, 2)).astype(ml_dtypes.bfloat16)
    Ut = np.ascontiguousarray(
        U.reshape(KT, 128, G4).transpose(1, 0, 2)
    ).astype(ml_dtypes.bfloat16)
    bt = np.ascontiguousarray(bias.reshape(MT, 128).T)

    in_maps = []
    for i in range(NC):
        xl = x[i * BL:(i + 1) * BL]                     # [8, 1024, 512]
        xTl = np.ascontiguousarray(
            xl.transpose(2, 1, 0).reshape(KT, 128, C)   # [512, T, 8]->[4,128,C]
        ).transpose(1, 0, 2)
        in_maps.append({
            "xT": np.ascontiguousarray(xTl).astype(ml_dtypes.bfloat16),
            "W": Wt, "U": Ut, "biasT": bt,
            "ident": np.eye(128, dtype=ml_dtypes.bfloat16),
        })

    import os
    trace = bool(os.environ.get("LSTM_TRACE"))
    res = run_bass_kernel_spmd(
        nc, in_maps, core_ids=list(range(NC)), trace=trace
    )
    if trace and res.exec_time_ns is not None:
        print(f"HW exec time: {res.exec_time_ns} ns")
        print("trace:", (res.instructions_and_trace or (None, None))[1])
    out = np.empty((B, T, H), np.float32)
    for i in range(NC):
        ho = np.asarray(res.results[i]["hT_out"]).astype(np.float32)  # [128, 4, C]
        out[i * BL:(i + 1) * BL] = (
            ho.reshape(128, KT, T, BL).transpose(3, 2, 1, 0).reshape(BL, T, H)
        )
    return out
